# revision 7
# baseline (speedup 1.0000x reference)
"""Trainium2 Bass kernel for fused MHA with q/k std-normalization.

Math restructure vs v1:
- Wq/Wk are CENTERED over e on the host (W -= mean_e W), so q,k projections
  are born mean-free: std-normalization reduces to a pure scaling by
  inv = 1/sqrt(sum(q^2)/63)  (the +1e-5 on sigma is dropped; |effect| ~2e-5).
- q,k are projected TRANSPOSED (W stationary, x moving) directly into
  [e, t] layout -- no PE transposes, no bn_stats.
- sum(q^2) over e (a partition reduction) via ACT Square -> selector-matmul
  into a [4, 512] psum tile; inv = Exp(-0.5*Log(S) + 0.5*ln 63) using the
  natural_log_exp table set => zero ACT table switches.
- inv scaling applied during the single DVE evacuation of each proj tile
  (rep-matmul broadcasts inv rows across partitions).
- scores pair = the two heads of a head-pair in disjoint PE row groups;
  probs tile [s 128, 2 heads x 512 t]; one exp (const scale 1/8) per tile.
- v projection and attention / ones-column softmax denominator as v1.

Sharding: 8 cores = 4 batches x 2 half-head-groups (as v1).

Scheduling: every cross-engine scalar chain is emitted one block late so
the PE's strict program-order matmul queue never waits on an in-flight
ACT/DVE result:
- proj stats (sums-MM -> Ln -> Exp) flush during the NEXT head-pair's
  q-chain; evacuations (rep-MM -> repS -> TT) during its k-chain;
- softmax reciprocals are strip-batched (one Ln + one Exp over all 8
  heads' denominators) with the broadcast matmul + multiply + DMA
  deferred into the following strip's units;
- SBUF pools are hoisted above the PSUM phase pools so next-iteration
  input DMAs overlap the current iteration's attention (For_i bench).

PSUM budget (8 banks): phase P: pv 2 + pqk 4 + sums 1 + rep 1 = 8;
phase A: sp 2x2 + op 2 + divrep 2 = 8.
"""

import sys

if "/opt/trn_rl_repo" not in sys.path:
    sys.path.insert(0, "/opt/trn_rl_repo")

import math
import numpy as np

B, D, T, H = 4, 1024, 2048, 16
NHL = 8            # heads per core
NHP = 4            # head pairs per core
DH = 64
NT = T // 128      # 16 s-chunks
ND = D // 128      # 8 d-chunks
NST = T // 512     # 4 t-strips

_prog = None


def _build(loop_n=None, part=None):
    import contextlib
    import concourse.bass as bass
    import concourse.bacc as bacc
    import concourse.tile as tile
    from concourse import mybir

    f32 = mybir.dt.float32
    f32r = mybir.dt.float32r
    bf16 = mybir.dt.bfloat16
    i16 = mybir.dt.int16
    AF = mybir.ActivationFunctionType
    ALU = mybir.AluOpType
    # Schraudolph exp on DVE for a subset of s-chunks: probs = exp(s/8)
    # ~= bf16_bits(round(s * (16/ln2) + 16256 - 7)); max rel err ~3-4%,
    # validated end-to-end ~5e-3 extra output error at 6/16 coverage.
    SCH_A = float(16.0 / math.log(2.0))
    SCH_B = 16256.0 - 7.0
    DVE_CHUNKS = ()

    nc = bacc.Bacc()
    x_ext = nc.dram_tensor("x_local", [ND, 128, T], bf16, kind="ExternalInput")
    w_ext = nc.dram_tensor("w_local", [ND, 128, 1536], bf16,
                           kind="ExternalInput")
    sel_ext = nc.dram_tensor("sel_local", [128, 264], bf16,
                             kind="ExternalInput")
    rsel_ext = nc.dram_tensor("rsel_local", [4, 256], f32r,
                              kind="ExternalInput")
    out_ext = nc.dram_tensor("out_local", [NHL * DH, T], f32,
                             kind="ExternalOutput")

    LN63H = 0.5 * math.log(63.0)

    with tile.TileContext(nc) as tc:
      with (tc.For_i(0, loop_n, 1) if loop_n else contextlib.nullcontext()):
        with tc.tile_pool(name="persist", bufs=1) as persist, \
             tc.tile_pool(name="qkh", bufs=1) as qkh, \
             tc.tile_pool(name="vp", bufs=1) as vp:
            ones64 = persist.tile([1, 64], f32r, tag="ones64")
            nc.vector.memset(ones64.bitcast(f32), 1.0)
            # selector constants, DMA'd from host:
            # sqsel[qk] = sel[:, qk*4:(qk+1)*4]   ([128, 4] one-hot blocks)
            # repsel[qk] = sel[0:4, 8+qk*128 : 8+(qk+1)*128]
            sel = persist.tile([128, 264], bf16, tag="sel")
            nc.sync.dma_start(out=sel, in_=sel_ext[:, :])
            sqsel = [sel[:, 0:4], sel[:, 4:8]]
            rsel = persist.tile([4, 256], f32r, tag="rsel")
            nc.sync.dma_start(out=rsel, in_=rsel_ext[:, :])
            repsel = [rsel[:, 0:128], rsel[:, 128:256]]

            qhat = [qkh.tile([128, T], bf16, tag=f"qh{hp}", name=f"qh{hp}")
                    for hp in range(NHP)]
            khat = [qkh.tile([128, T], bf16, tag=f"kh{hp}", name=f"kh{hp}")
                    for hp in range(NHP)]
            vt = vp.tile([128, NT, NHL, 65], bf16, tag="vt", name="vt")
            nc.gpsimd.memset(vt[:, :, :, 64:65], 1.0)

            with tc.tile_pool(name="xw", bufs=1) as xw, \
                 tc.tile_pool(name="sqs", bufs=4) as sqs, \
                 tc.tile_pool(name="stat", bufs=4) as statp, \
                 tc.tile_pool(name="reps", bufs=4) as repsp:
              _pstack = contextlib.ExitStack()
              pp = _pstack.enter_context(
                  tc.tile_pool(name="pp", bufs=2, space="PSUM"))
              sup = _pstack.enter_context(
                  tc.tile_pool(name="sums", bufs=1, space="PSUM"))
              repp = _pstack.enter_context(
                  tc.tile_pool(name="repp", bufs=1, space="PSUM"))
              if True:
                xq = [xw.tile([128, T], bf16, tag=f"x{dc}", name=f"x{dc}")
                      for dc in range(ND)]
                wsb = [xw.tile([128, 1536], bf16, tag=f"w{dc}", name=f"w{dc}")
                       for dc in range(ND)]
                for dc in range(ND):
                    nc.sync.dma_start(out=wsb[dc], in_=w_ext[dc])
                for dc in range(ND):
                    nc.sync.dma_start(out=xq[dc][:, 0:512],
                                      in_=x_ext[dc][:, 0:512])

                # -------- Phase P: projections, per t/s-strip.
                # Stats (sums-MM -> Ln -> Exp) and evacuation (rep-MM ->
                # repS -> TT) chains are emitted one half/full block late so
                # the PE never waits on an in-flight ACT result.
                pend_stats = []
                pend_evac = []

                def flush_stats():
                    while pend_stats:
                        hp2, st2, pst2, sq2 = pend_stats.pop(0)
                        sums = sup.tile([4, 512], f32, tag="sums")
                        for qk in range(2):
                            nc.tensor.matmul(
                                sums, lhsT=sqsel[qk], rhs=sq2[qk],
                                start=(qk == 0), stop=(qk == 1))
                        lnt = statp.tile([4, 512], f32, tag="lnt")
                        nc.scalar.activation(lnt, sums, AF.Ln,
                                             scale=float(1.0 / 63.0))
                        inv = statp.tile([4, 512], f32r, tag="inv")
                        nc.scalar.activation(inv, lnt, AF.Exp, scale=-0.5)
                        pend_evac.append((hp2, st2, pst2, inv))

                def flush_evac():
                    while pend_evac:
                        hp2, st2, pst2, inv2 = pend_evac.pop(0)
                        for qk in range(2):
                            rep = repp.tile([128, 512], f32, tag="rep")
                            nc.tensor.matmul(
                                rep, lhsT=repsel[qk], rhs=inv2,
                                start=True, stop=True)
                            repS = repsp.tile([128, 512], f32, tag="repS")
                            nc.vector.tensor_copy(repS, rep)
                            dst = (qhat if qk == 0 else khat)[hp2]
                            nc.vector.tensor_mul(
                                dst[:, st2 * 512:st2 * 512 + 512],
                                pst2[qk], repS)

                for st in range(NST):
                    if st + 1 < NST:
                        lo = (st + 1) * 512
                        for dc in range(ND):
                            nc.sync.dma_start(
                                out=xq[dc][:, lo:lo + 512],
                                in_=x_ext[dc][:, lo:lo + 512])
                    # v-projection for this strip's 4 s-tiles
                    for ti in range(4 * st, 4 * st + 4):
                        psv = pp.tile([128, 512], f32, tag="pv")
                        for dc in range(ND):
                            nc.tensor.matmul(
                                psv,
                                lhsT=xq[dc][:, ti * 128:(ti + 1) * 128],
                                rhs=wsb[dc][:, 0:512],
                                start=(dc == 0), stop=(dc == ND - 1))
                        nc.vector.tensor_copy(vt[:, ti, :, 0:64], psv)
                    # q/k projections per head-pair, staged emission
                    for hp in range(NHP):
                        pst = {}
                        sqd = {}
                        for qk in range(2):
                            ps = pp.tile([128, 512], f32, tag="pqk", bufs=4)
                            wlo = 512 + qk * 512 + hp * 128
                            for dc in range(ND):
                                nc.tensor.matmul(
                                    ps,
                                    lhsT=wsb[dc][:, wlo:wlo + 128],
                                    rhs=xq[dc][:, st * 512:st * 512 + 512],
                                    start=(dc == 0), stop=(dc == ND - 1))
                            pst[qk] = ps
                            sq = sqs.tile([128, 512], bf16, tag="sq")
                            nc.scalar.activation(sq, ps, AF.Square)
                            sqd[qk] = sq
                            if qk == 0:
                                flush_stats()
                            else:
                                flush_evac()
                        pend_stats.append((hp, st, pst, sqd))
                flush_stats()
                flush_evac()

            # -------- Phase A: attention
            _pstack.close()
            if part != "pP":
              with tc.tile_pool(name="spp", bufs=2, space="PSUM") as spp, \
                   tc.tile_pool(name="opp", bufs=2, space="PSUM") as opp, \
                   tc.tile_pool(name="trp", bufs=2, space="PSUM") as trp, \
                   tc.tile_pool(name="ptp", bufs=6) as ptp, \
                   tc.tile_pool(name="osb", bufs=1) as osbp, \
                   tc.tile_pool(name="dt", bufs=1) as dtp, \
                   tc.tile_pool(name="outsb", bufs=6) as outp:
                pend_fin = []
                osbS = {"cur": None}

                def emit_div(h, st, op):
                    # Free the op bank with one copy into the strip tile.
                    # The reciprocal work is batched per strip (emit_stats).
                    if osbS["cur"] is None:
                        osbS["cur"] = osbp.tile([65, 8, 512], f32,
                                                tag="osb", name="osbS")
                    nc.vector.tensor_copy(osbS["cur"][:, h, :], op)

                def emit_stats(st):
                    # One Ln + one Exp for all 8 heads' denominators (the
                    # DVE reciprocal is ~8 cyc/elem on one partition; and
                    # per-unit small ACT ops serialize against the PE).
                    osb = osbS["cur"]
                    osbS["cur"] = None
                    lnr = dtp.tile([1, 8, 512], f32, tag="rtf")
                    nc.scalar.activation(lnr, osb[64:65, :, :], AF.Ln)
                    rtS = dtp.tile([1, 8, 512], f32r, tag="rt", bufs=2)
                    nc.scalar.activation(rtS, lnr, AF.Exp, scale=-1.0)
                    for h in range(NHL):
                        pend_fin.append((h, st, osb, rtS))

                def emit_fin(n=99):
                    # Deferred into the NEXT strip's units: the broadcast
                    # matmul blocks the PE's strict program order until rtS
                    # is ready, so emit it long after the strip's stats.
                    for _ in range(min(n, len(pend_fin))):
                        h, st, osb, rtS = pend_fin.pop(0)
                        rep = trp.tile([64, 512], f32, tag="rrep")
                        nc.tensor.matmul(rep, lhsT=ones64, rhs=rtS[:, h, :],
                                         start=True, stop=True)
                        outt = outp.tile([64, 512], f32, tag="outt")
                        nc.vector.tensor_mul(outt, osb[0:64, h, :], rep)
                        nc.sync.dma_start(
                            out=out_ext[h * 64:(h + 1) * 64,
                                        st * 512:st * 512 + 512],
                            in_=outt)

                pend_div = []
                for st in range(NST):
                    for hp in range(NHP):
                        ops = [opp.tile([65, 512], f32, tag="op", name="op")
                               for _ in range(2)]
                        pts = []

                        def emit_scores2(scp, hp=hp, st=st):
                            # scores for s-chunk pair (2*scp, 2*scp+1) into
                            # one [128, 2048] psum tile (4 matmuls)
                            sp = spp.tile([128, 2048], f32, tag="sp")
                            for half in range(2):
                                for u in range(2):
                                    lo = half * 1024 + u * 512
                                    nc.tensor.matmul(
                                        sp[:, lo:lo + 512],
                                        lhsT=khat[hp][
                                            u * 64:u * 64 + 64,
                                            (2 * scp + half) * 128:
                                            (2 * scp + half + 1) * 128],
                                        rhs=qhat[hp][u * 64:u * 64 + 64,
                                                     st * 512:
                                                     st * 512 + 512],
                                        start=True, stop=True)
                            return sp

                        def emit_exp2(sp, pts=pts):
                            pt = ptp.tile([128, 2048], bf16, tag="pt")
                            nc.scalar.activation(pt, sp, AF.Exp, scale=0.125)
                            pts.append(pt)

                        def emit_pv2(scp, ops=ops, pts=pts, hp=hp):
                            pt = pts.pop(0)
                            for half in range(2):
                                sc = 2 * scp + half
                                for u in range(2):
                                    lo = half * 1024 + u * 512
                                    nc.tensor.matmul(
                                        ops[u],
                                        lhsT=vt[:, sc, 2 * hp + u, :],
                                        rhs=pt[:, lo:lo + 512],
                                        start=(sc == 0),
                                        stop=(sc == NT - 1))

                        sps = []

                        def emit_scores(sc, hp=hp, st=st, sps=sps):
                            sp = spp.tile([128, 1024], f32, tag="sp")
                            for u in range(2):
                                nc.tensor.matmul(
                                    sp[:, u * 512:(u + 1) * 512],
                                    lhsT=khat[hp][u * 64:u * 64 + 64,
                                                  sc * 128:(sc + 1) * 128],
                                    rhs=qhat[hp][u * 64:u * 64 + 64,
                                                 st * 512:st * 512 + 512],
                                    start=True, stop=True)
                            sps.append(sp)

                        def emit_exp(sps=sps, pts=pts):
                            sp = sps.pop(0)
                            pt = ptp.tile([128, 1024], bf16, tag="pt")
                            nc.scalar.activation(pt, sp, AF.Exp, scale=0.125)
                            pts.append(pt)

                        def emit_pv(sc, ops=ops, pts=pts, hp=hp):
                            pt = pts.pop(0)
                            for u in range(2):
                                nc.tensor.matmul(
                                    ops[u], lhsT=vt[:, sc, 2 * hp + u, :],
                                    rhs=pt[:, u * 512:(u + 1) * 512],
                                    start=(sc == 0), stop=(sc == NT - 1))

                        emit_scores(0)
                        emit_exp()
                        emit_scores(1)
                        for sc in range(NT):
                            if sc + 2 < NT:
                                emit_scores(sc + 2)
                            if sc + 1 < NT:
                                emit_exp()
                            emit_pv(sc)
                            if sc in (4, 8, 11, 14):
                                emit_fin(2)
                        emit_div(2 * hp, st, ops[0])
                        emit_div(2 * hp + 1, st, ops[1])
                        if hp == NHP - 1:
                            emit_stats(st)
                emit_fin()
    nc.finalize()
    return nc


def _get_prog():
    global _prog
    if _prog is None:
        _prog = _build()
    return _prog


def _make_sel():
    import ml_dtypes

    sel = np.zeros((128, 264), np.float32)
    rsel = np.zeros((4, 256), np.float32)
    for qk in range(2):
        for h01 in range(2):
            j = 2 * qk + h01
            sel[h01 * 64:(h01 + 1) * 64, qk * 4 + j] = 1.0
            rsel[j, qk * 128 + h01 * 64: qk * 128 + (h01 + 1) * 64] = 1.0
    return sel.astype(ml_dtypes.bfloat16), rsel


def make_in_maps(x, qkv):
    import ml_dtypes

    x = np.ascontiguousarray(np.asarray(x, dtype=np.float32))
    qkv = np.ascontiguousarray(np.asarray(qkv, dtype=np.float32))
    selb, rselb = _make_sel()
    in_maps = []
    for c in range(8):
        b = c // 2
        hb = (c % 2) * 8
        xp = x[b].reshape(ND, 128, T).astype(ml_dtypes.bfloat16)
        a = qkv[:, hb:hb + 8]              # [3(q,k,v), 8, D, 64]
        wq = a[0] - a[0].mean(axis=2, keepdims=True)
        wk = a[1] - a[1].mean(axis=2, keepdims=True)
        wv = a[2]
        wp = np.empty((D, 1536), np.float32)
        wp[:, 0:512] = wv.transpose(1, 0, 2).reshape(D, 512)
        wp[:, 512:1024] = wq.transpose(1, 0, 2).reshape(D, 512)
        wp[:, 1024:1536] = wk.transpose(1, 0, 2).reshape(D, 512)
        in_maps.append({"x_local": xp,
                        "w_local": wp.reshape(ND, 128, 1536)
                        .astype(ml_dtypes.bfloat16),
                        "sel_local": selb,
                        "rsel_local": rselb})
    return in_maps


def gather(results):
    out = np.empty((B, D, T), np.float32)
    for c in range(8):
        out[c // 2, (c % 2) * 512:(c % 2) * 512 + 512, :] = \
            results[c]["out_local"]
    return out


def kernel(**inputs):
    from concourse.bass_utils import run_bass_kernel_spmd

    nc = _get_prog()
    in_maps = make_in_maps(inputs["x"], inputs["qkv"])
    res = run_bass_kernel_spmd(nc, in_maps, list(range(8)))
    return gather(res.results)
